# revision 1
# baseline (speedup 1.0000x reference)
"""Causal self-attention on 8 trn2 NeuronCores.

Sharding: core c -> (batch b = c//2, head-group g = c%2).  Each head-group
is 8 heads = 512 channels.  Per core, one per-window pipeline:
  proj(w) -> out-proj(w-1) -> attention(w), with out-proj(3) at the end,
so the tensor queue never waits on a just-finished attention round.

  - q/k/v projections in bf16 (8 contraction chunks of 128).
  - QK^T as two concurrent row-tiled K=64 matmuls (heads 2j / 2j+1 live
    in partitions 0-63 / 64-127 of qT/kT); outputs land in separate PSUM
    banks of one [P, 2, W] tile.
  - exp on ScalarE into bf16 tiles [P, a, W], one per tk-tile, sized to
    the tile's valid (causal) query range.
  - AV accumulates per tk-tile into a [65, W] PSUM bank per head; each
    tile only streams its valid range (PSUM has_written handles the
    partial-region accumulation).  A ones-column per head in vp yields
    the softmax denominators for free.
  - causal masking multiplies each diagonal tile's [P, P] block by a
    bf16 triangular mask.
  - normalization: one [65, W] copy releases PSUM, then reciprocal +
    gpsimd partition-broadcast + multiply into yT.
  - output projection bf16, stored f16; host sums the two partials per
    batch and adds (bv @ Wo + bo).
"""

import numpy as np
import ml_dtypes

import concourse.bass as bass
import concourse.mybir as mybir
from concourse import bacc, tile
from concourse.bass_utils import run_bass_kernel_spmd

B, T, C, H = 4, 2048, 1024, 16
HD = C // H          # 64
G = 2                # head groups (cores per batch)
HG = H // G          # 8 heads per group
CG = C // G          # 512 channels per group
CGP = CG // 128      # 4 c_out tiles per group
P = 128
W = 512              # free-dim window (one PSUM bank of f32)
NW = T // W          # 4 windows
NTT = T // P         # 16 t tiles
NCI = C // P         # 8 c_in chunks
VS = HD + 1          # 65: v plus ones column

_cached_nc = None


def _build():
    f32 = mybir.dt.float32
    f16 = mybir.dt.float16
    bf16 = mybir.dt.bfloat16
    AF = mybir.ActivationFunctionType
    nc = bacc.Bacc("TRN2", target_bir_lowering=False, debug=False, num_devices=8)

    xt_d = nc.dram_tensor("xt", [C, T], bf16, kind="ExternalInput")
    wq_d = nc.dram_tensor("wq", [C, CG], bf16, kind="ExternalInput")
    wk_d = nc.dram_tensor("wk", [C, CG], bf16, kind="ExternalInput")
    wv_d = nc.dram_tensor("wv", [C, CG], bf16, kind="ExternalInput")
    wo_d = nc.dram_tensor("wo", [CG, C], bf16, kind="ExternalInput")
    bq_d = nc.dram_tensor("bq", [P, CGP], f32, kind="ExternalInput")
    bk_d = nc.dram_tensor("bk", [P, CGP], f32, kind="ExternalInput")
    mask_d = nc.dram_tensor("maskT", [P, P], bf16, kind="ExternalInput")
    out_d = nc.dram_tensor("outp", [C, T], f16, kind="ExternalOutput")

    def mm(out, lhsT, rhs, start, stop, **kw):
        return nc.tensor.matmul(out, lhsT, rhs, start=start, stop=stop, **kw)

    escale = 1.0 / float(np.sqrt(HD))

    with tile.TileContext(nc) as tc:
        with tc.tile_pool(name="pers", bufs=1) as pers:
            qT = pers.tile([P, CGP, T], bf16)
            kT = pers.tile([P, CGP, T], bf16)
            yT = pers.tile([P, CGP, T], bf16)
            vp = pers.tile([P, NTT, HG * VS], bf16)
            wo_sb = pers.tile([P, CGP, C], bf16)
            wh = pers.tile([P, 3, NCI, CG], bf16)
            xc = pers.tile([P, NCI, T], bf16)
            bq_sb = pers.tile([P, CGP], f32)
            bk_sb = pers.tile([P, CGP], f32)
            maskT = pers.tile([P, P], bf16)

            # ones columns of vp (v writes later overwrite the v lanes)
            nc.vector.memset(vp, 1.0)

            # ---- DMAs, in the order compute needs them ----
            nc.sync.dma_start(out=bq_sb, in_=bq_d.ap())
            nc.sync.dma_start(out=bk_sb, in_=bk_d.ap())
            nc.sync.dma_start(out=maskT, in_=mask_d.ap())
            # first q-projection (j=0) starts after one wq block + one x chunk
            nc.sync.dma_start(
                out=wh[:, 0, :, 0:P],
                in_=wq_d.ap()[:, 0:P].rearrange("(c p) n -> p c n", p=P))
            for i in range(NCI):
                nc.sync.dma_start(
                    out=xc[:, i, 0:W],
                    in_=xt_d.ap()[i * P:(i + 1) * P, 0:W])
            for j in range(1, CGP):
                nc.sync.dma_start(
                    out=wh[:, 0, :, j * P:(j + 1) * P],
                    in_=wq_d.ap()[:, j * P:(j + 1) * P]
                        .rearrange("(c p) n -> p c n", p=P))
            nc.sync.dma_start(
                out=wh[:, 1], in_=wk_d.ap().rearrange("(c p) n -> p c n", p=P))
            nc.sync.dma_start(
                out=wh[:, 2], in_=wv_d.ap().rearrange("(c p) n -> p c n", p=P))
            nc.sync.dma_start(
                out=wo_sb, in_=wo_d.ap().rearrange("(c p) n -> p c n", p=P))
            for w in range(1, NW):
                ws = slice(w * W, (w + 1) * W)
                nc.sync.dma_start(
                    out=xc[:, :, ws],
                    in_=xt_d.ap()[:, ws].rearrange("(i p) t -> p i t", p=P))

            with (
                tc.tile_pool(name="ppa", bufs=2, space="PSUM") as ppa,
                tc.tile_pool(name="pqk", bufs=2, space="PSUM") as pqk,
                tc.tile_pool(name="pav", bufs=2, space="PSUM") as pav,
                tc.tile_pool(name="ptp", bufs=8) as ptp,
                tc.tile_pool(name="avs", bufs=4) as avsp,
                tc.tile_pool(name="dnp", bufs=4) as dnp,
                tc.tile_pool(name="rbp", bufs=4) as rbp,
                tc.tile_pool(name="otp", bufs=3) as otp,
            ):
                def proj_groups(w):
                    """Closures, each one psum round: q/k j-block or v t-tile."""
                    ws = slice(w * W, (w + 1) * W)

                    def qk_group(wi, dst, b_sb, j):
                        def g():
                            ps = ppa.tile([P, W], f32, tag="pp", name="psqj")
                            for i in range(NCI):
                                mm(ps, wh[:, wi, i, j * P:(j + 1) * P],
                                   xc[:, i, ws],
                                   start=(i == 0), stop=(i == NCI - 1))
                            nc.vector.tensor_scalar_add(
                                dst[:, j, ws], ps, b_sb[:, j:j + 1])
                        return g

                    def v_group(it):
                        def g():
                            ps = ppa.tile([P, W], f32, tag="pp", name="psvt")
                            for i in range(NCI):
                                mm(ps, xc[:, i, it * P:(it + 1) * P],
                                   wh[:, 2, i, :],
                                   start=(i == 0), stop=(i == NCI - 1))
                            vdst = (vp[:, it, :]
                                    .rearrange("p (h x) -> p h x",
                                               x=VS)[:, :, 0:HD])
                            nc.vector.tensor_copy(
                                vdst, ps.rearrange("p (h x) -> p h x", x=HD))
                        return g

                    gs = []
                    for wi, dst, b_sb in ((0, qT, bq_sb), (1, kT, bk_sb)):
                        for j in range(CGP):
                            gs.append(qk_group(wi, dst, b_sb, j))
                    for it in range(4 * w, 4 * w + 4):
                        gs.append(v_group(it))
                    return gs

                def oproj_groups(w):
                    ws = slice(w * W, (w + 1) * W)

                    def o_group(m):
                        def g():
                            po = ppa.tile([P, W], f32, tag="pp", name="po")
                            for i in range(CGP):
                                mm(po, wo_sb[:, i, m * P:(m + 1) * P],
                                   yT[:, i, ws],
                                   start=(i == 0), stop=(i == CGP - 1))
                            ot = otp.tile([P, W], f16, tag="ot")
                            nc.vector.tensor_copy(ot, po)
                            nc.sync.dma_start(
                                out=out_d.ap()[m * P:(m + 1) * P, ws], in_=ot)
                        return g

                    return [o_group(m) for m in range(C // P)]

                # prologue: window-0 projections run un-interleaved
                for g in proj_groups(0):
                    g()

                for w in range(NW):
                    ws = slice(w * W, (w + 1) * W)
                    tq0 = w * W
                    ntk = 4 * (w + 1)

                    # tensor work to interleave into attention idle slots:
                    # out-proj of the previous window + projections of the next
                    work = []
                    if w > 0:
                        work += oproj_groups(w - 1)
                    if w < NW - 1:
                        work += proj_groups(w + 1)
                    nwork = len(work)
                    total_tiles = CGP * ntk
                    seen = 0
                    emitted = 0

                    for j in range(CGP):
                        ps_av = [pav.tile([VS, W], f32, tag="av",
                                          name=f"av{w}_{j}_{a}")
                                 for a in range(2)]
                        for i in range(ntk):
                            nn = tq0 + W - max(tq0, i * P)
                            c0 = W - nn
                            psqk = pqk.tile([P, 2, W], f32, tag="qk")
                            for a in range(2):
                                mm(psqk[:, a, c0:],
                                   kT[64 * a:64 * (a + 1), j,
                                      i * P:(i + 1) * P],
                                   qT[64 * a:64 * (a + 1), j,
                                      tq0 + c0:tq0 + W],
                                   start=True, stop=True)
                            pt_t = ptp.tile([P, 2, W], bf16, tag="pt")
                            nc.scalar.activation(
                                pt_t[:, :, c0:], psqk[:, :, c0:],
                                AF.Exp, scale=escale)
                            if i * P >= tq0:  # diagonal tile: mask its block
                                for a in range(2):
                                    nc.vector.tensor_mul(
                                        pt_t[:, a, c0:c0 + P],
                                        pt_t[:, a, c0:c0 + P], maskT)
                            for a in range(2):
                                h = 2 * j + a
                                mm(ps_av[a][:, c0:],
                                   vp[:, i, h * VS:(h + 1) * VS],
                                   pt_t[:, a, c0:],
                                   start=(i == 0), stop=(i == ntk - 1))
                            seen += 1
                            target = (nwork * seen) // total_tiles
                            while emitted < target:
                                work[emitted]()
                                emitted += 1
                        for a in range(2):
                            av_sb = avsp.tile([VS, W], f32, tag="avs")
                            nc.vector.tensor_copy(av_sb, ps_av[a])
                            dn = dnp.tile([1, W], f32, tag="dn")
                            nc.vector.tensor_copy(dn, av_sb[HD:VS, :])
                            nc.vector.reciprocal_approx_fast(out=dn, in_=dn)
                            rb = rbp.tile([HD, W], f32, tag="rb")
                            nc.gpsimd.partition_broadcast(rb, dn)
                            nc.vector.tensor_mul(
                                yT[64 * a:64 * (a + 1), j, ws],
                                av_sb[0:HD, :], rb)
                    while emitted < nwork:
                        work[emitted]()
                        emitted += 1

                for g in oproj_groups(NW - 1):
                    g()

    nc.compile()
    return nc


def get_nc():
    global _cached_nc
    if _cached_nc is None:
        _cached_nc = _build()
    return _cached_nc


def make_in_maps(x, Wq, bq, Wk, bk, Wv, bv, Wo, bo):
    x = np.asarray(x, np.float32)
    tri = np.triu(np.ones((P, P), np.float32))
    in_maps = []
    for c in range(8):
        b, g = c // 2, c % 2
        cs = slice(g * CG, (g + 1) * CG)
        in_maps.append({
            "xt": np.ascontiguousarray(x[b].T.astype(ml_dtypes.bfloat16)),
            "wq": np.ascontiguousarray(
                np.asarray(Wq, np.float32)[:, cs].astype(ml_dtypes.bfloat16)),
            "wk": np.ascontiguousarray(
                np.asarray(Wk, np.float32)[:, cs].astype(ml_dtypes.bfloat16)),
            "wv": np.ascontiguousarray(
                np.asarray(Wv, np.float32)[:, cs].astype(ml_dtypes.bfloat16)),
            "wo": np.ascontiguousarray(
                np.asarray(Wo, np.float32)[cs, :].astype(ml_dtypes.bfloat16)),
            "bq": np.ascontiguousarray(
                np.asarray(bq, np.float32)[cs].reshape(CGP, P).T),
            "bk": np.ascontiguousarray(
                np.asarray(bk, np.float32)[cs].reshape(CGP, P).T),
            "maskT": tri.astype(ml_dtypes.bfloat16),
        })
    return in_maps


def combine(results, Wv, bv, Wo, bo):
    const = (np.asarray(bv, np.float32) @ np.asarray(Wo, np.float32)
             + np.asarray(bo, np.float32))
    out = np.empty((B, T, C), np.float32)
    for b in range(B):
        acc = (results[2 * b]["outp"].astype(np.float32)
               + results[2 * b + 1]["outp"].astype(np.float32))
        out[b] = acc.T + const[None, :]
    return out


def kernel(x, Wq, bq, Wk, bk, Wv, bv, Wo, bo):
    nc = get_nc()
    in_maps = make_in_maps(x, Wq, bq, Wk, bk, Wv, bv, Wo, bo)
    res = run_bass_kernel_spmd(nc, in_maps, core_ids=list(range(8)))
    return combine(res.results, Wv, bv, Wo, bo)

